# revision 20
# baseline (speedup 1.0000x reference)
"""Paged-attention decode kernel for 8 TRN2 NeuronCores (SPMD, data-parallel over sequences).

Problem: nn_Attention_15659450761267 (sparse_attention).
  S=64 seqs, H=32 query heads, HKV=8 kv heads (GQA g=4), D=128, BS=16,
  MAX_BLOCKS=128, T=2048, f32 caches [8192,16,8,128].

Sharding: core c owns sequences [8c, 8c+8). block_tables is arange
(spec fill), so sequence s's cache lives in blocks [128s, 128(s+1)) ->
its K/V cache is a contiguous [2048, 1024] f32 slab. Each core reads
only its own 8 slabs (134 MB) -> memory-roofline ~375us/core.

The reference scatters the new-token k/v into the cache at slot cl-1,
then attends over positions < cl. Equivalently (softmax is permutation
invariant): attend over cached positions t < cl-1 (masking out the
stale slot cl-1) plus the new (k, v) appended as an extra column.
No device-side scatter needed.
"""

import numpy as np

S = 64
H = 32
HKV = 8
G = H // HKV  # 4
D = 128
BS = 16
MAX_BLOCKS = 128
T = MAX_BLOCKS * BS  # 2048
SCALE = 0.08838834764831845
NCORES = 8
S_LOC = S // NCORES  # 8
NEG = -1.0e30
CHUNK = 128          # positions per K/V chunk (one transpose block)
NCHUNK = T // CHUNK  # 16
BLK = 512            # positions per score-matmul block (fp32r full rate needs N>=256)
NBLK = T // BLK      # 4

_cached_nc = None


def _build_nc():
    import concourse.mybir as mybir
    import concourse.tile as tile
    from concourse import bacc
    from concourse.masks import make_identity

    f32 = mybir.dt.float32
    f32r = mybir.dt.float32r
    i32 = mybir.dt.int32
    Alu = mybir.AluOpType
    Act = mybir.ActivationFunctionType
    Ax = mybir.AxisListType

    nc = bacc.Bacc("TRN2", target_bir_lowering=False, debug=False,
                   num_devices=NCORES)
    q_d = nc.dram_tensor("q", [S_LOC, H, D], f32, kind="ExternalInput")
    k_d = nc.dram_tensor("k", [S_LOC, HKV, D], f32, kind="ExternalInput")
    v_d = nc.dram_tensor("v", [S_LOC, HKV, D], f32, kind="ExternalInput")
    kc_d = nc.dram_tensor("kc", [S_LOC, T, HKV * D], f32, kind="ExternalInput")
    vc_d = nc.dram_tensor("vc", [S_LOC, T, HKV * D], f32, kind="ExternalInput")
    cl_d = nc.dram_tensor("cl", [1, S_LOC], i32, kind="ExternalInput")
    out_d = nc.dram_tensor("out", [S_LOC, H, D], f32, kind="ExternalOutput")

    with tile.TileContext(nc) as tc:
        with (
            tc.tile_pool(name="const", bufs=1) as constp,
            tc.tile_pool(name="kchunk", bufs=3) as kpool,
            tc.tile_pool(name="vchunk", bufs=3) as vpool,
            tc.tile_pool(name="kt", bufs=2) as ktpool,
            tc.tile_pool(name="scores", bufs=2) as scpool,
            tc.tile_pool(name="small", bufs=3) as smpool,
            tc.tile_pool(name="ps_ktp", bufs=2, space="PSUM") as ps_ktp,
            tc.tile_pool(name="ps_pt", bufs=2, space="PSUM") as ps_pt,
            tc.tile_pool(name="ps_sc", bufs=2, space="PSUM") as ps_sc,
            tc.tile_pool(name="ps_pv", bufs=1, space="PSUM") as ps_pv,
        ):
            ident = constp.tile([128, 128], f32)
            make_identity(nc, ident[:])

            # positions 0..T-1 replicated on 32 partitions (channel_multiplier=0)
            posb_i = constp.tile([H, T], i32)
            nc.gpsimd.iota(posb_i[:], pattern=[[1, T]], base=0,
                           channel_multiplier=0)
            posb = constp.tile([H, T], f32)
            nc.vector.tensor_copy(posb[:], posb_i[:])

            # context_lens -> f32 (cl - 1), broadcast over 32 partitions
            cli = constp.tile([1, S_LOC], i32)
            nc.sync.dma_start(cli[:], cl_d[:])
            clf = constp.tile([1, S_LOC], f32)
            nc.vector.tensor_copy(clf[:], cli[:])
            nc.vector.tensor_scalar_add(clf[:], clf[:], -1.0)
            clb = constp.tile([H, S_LOC], f32)
            nc.gpsimd.partition_broadcast(clb[:], clf[:])

            for s in range(S_LOC):
                # ---- q / new-token k,v ----
                q_sb = smpool.tile([H, D], f32, tag="q")
                nc.sync.dma_start(q_sb[:], q_d[s])
                kn_sb = smpool.tile([HKV, D], f32, tag="kn")
                nc.sync.dma_start(kn_sb[:], k_d[s])
                vn_sb = smpool.tile([1, HKV * D], f32r, tag="vn")
                nc.gpsimd.dma_start(
                    vn_sb[:], v_d.rearrange("s h d -> s (h d)")[s][None, :])

                # QT = q^T * SCALE  [D, H]
                qt_ps = ps_pt.tile([D, H], f32, tag="ptq")
                nc.tensor.transpose(qt_ps[:], q_sb[:], ident[:H, :H])
                qt_sb = smpool.tile([D, H], f32r, tag="qt")
                nc.scalar.mul(qt_sb[:], qt_ps[:], SCALE)

                # ---- phase A: scores[h*4+g, t] = (q . k_t) * SCALE ----
                scores = scpool.tile([H, T + 1], f32, tag="scores")
                for b in range(NBLK):
                    kt = ktpool.tile([D, HKV, BLK], f32r, tag="kt")
                    for c2 in range(BLK // CHUNK):
                        c = b * (BLK // CHUNK) + c2
                        k_sb = kpool.tile([CHUNK, HKV * D], f32, tag="kchunk")
                        nc.sync.dma_start(
                            k_sb[:], kc_d[s, c * CHUNK:(c + 1) * CHUNK, :])
                        for h in range(HKV):
                            ktp = ps_ktp.tile([D, CHUNK], f32, tag="ktp")
                            nc.tensor.transpose(
                                ktp[:], k_sb[:, h * D:(h + 1) * D], ident[:])
                            dst = kt[:, h, c2 * CHUNK:(c2 + 1) * CHUNK]
                            if h % 2 == 0:
                                nc.vector.tensor_copy(dst, ktp[:])
                            else:
                                nc.scalar.copy(dst, ktp[:])
                    for h in range(HKV):
                        sc_ps = ps_sc.tile([G, BLK], f32, tag="sc")
                        nc.tensor.matmul(
                            sc_ps[:], qt_sb[:, G * h:G * (h + 1)],
                            kt[:, h], start=True, stop=True)
                        # engine ops need 32-aligned partition bases: stage at
                        # base 0, then SBUF->SBUF DMA into the 4-row band.
                        sc_st = smpool.tile([G, BLK], f32, tag="scstage")
                        nc.scalar.copy(sc_st[:], sc_ps[:])
                        nc.sync.dma_start(
                            scores[G * h:G * (h + 1), b * BLK:(b + 1) * BLK],
                            sc_st[:])

                # ---- new-token score column (qt_sb already carries SCALE) ----
                ktn_ps = ps_ktp.tile([D, HKV], f32, tag="ktp")
                nc.tensor.transpose(ktn_ps[:], kn_sb[:], ident[:HKV, :HKV])
                ktn_sb = smpool.tile([D, HKV], f32r, tag="ktn_sb")
                nc.vector.tensor_copy(ktn_sb[:], ktn_ps[:])
                scn_st = smpool.tile([G, HKV], f32, tag="scnstage")
                for h in range(HKV):
                    scn_ps = ps_sc.tile([G, HKV], f32, tag="sc")
                    nc.tensor.matmul(scn_ps[:], qt_sb[:, G * h:G * (h + 1)],
                                     ktn_sb[:], start=True, stop=True)
                    nc.scalar.copy(scn_st[:, h:h + 1], scn_ps[:, h:h + 1])
                for h in range(HKV):
                    nc.sync.dma_start(scores[G * h:G * (h + 1), T:T + 1],
                                      scn_st[:, h:h + 1])

                # ---- mask: positions >= cl-1 get -1e30 (stale slot + beyond) --
                mask = smpool.tile([H, T], f32, tag="mask")
                nc.vector.tensor_scalar(
                    mask[:], posb[:], clb[:, s:s + 1], NEG,
                    op0=Alu.is_ge, op1=Alu.mult)
                nc.vector.tensor_tensor(
                    scores[:, :T], scores[:, :T], mask[:], Alu.add)

                # ---- softmax (scores are O(+-8); exp without max-subtract) ----
                sums = smpool.tile([H, 1], f32, tag="sums")
                nc.scalar.activation(scores[:], scores[:], Act.Exp,
                                     accum_out=sums[:])
                rcp = smpool.tile([H, 1], f32, tag="rcp")
                nc.vector.reciprocal(rcp[:], sums[:])
                nc.vector.tensor_scalar_mul(scores[:], scores[:], rcp[:, 0:1])

                # ---- phase B: out = p @ V ----
                pv_ps = ps_pv.tile([H, HKV * D], f32, tag="pv")
                for c in range(NCHUNK):
                    v_sb = vpool.tile([CHUNK, HKV * D], f32r, tag="vchunk")
                    nc.gpsimd.dma_start(
                        v_sb[:], vc_d[s, c * CHUNK:(c + 1) * CHUNK, :])
                    pt_ps = ps_pt.tile([CHUNK, H], f32, tag="ptq")
                    nc.tensor.transpose(
                        pt_ps[:], scores[:, c * CHUNK:(c + 1) * CHUNK],
                        ident[:H, :H])
                    pt_sb = smpool.tile([CHUNK, H], f32r, tag="pt")
                    nc.vector.tensor_copy(pt_sb[:], pt_ps[:])
                    for half in range(2):
                        nc.tensor.matmul(
                            pv_ps[:, half * 512:(half + 1) * 512],
                            pt_sb[:],
                            v_sb[:, half * 512:(half + 1) * 512],
                            start=(c == 0), stop=False)

                # new-token V contribution: K=1 matmul appended to the group
                ptn_ps = ps_pt.tile([1, H], f32, tag="ptq")
                nc.tensor.transpose(ptn_ps[:], scores[:, T:T + 1],
                                    ident[:H, :H])
                ptn_sb = smpool.tile([1, H], f32r, tag="ptn_sb")
                nc.vector.tensor_copy(ptn_sb[:], ptn_ps[:])
                for half in range(2):
                    nc.tensor.matmul(
                        pv_ps[:, half * 512:(half + 1) * 512],
                        ptn_sb[:],
                        vn_sb[:, half * 512:(half + 1) * 512],
                        start=False, stop=True)

                # ---- stage PSUM at base-0 partitions, then DMA the diagonal
                # 4-row bands straight to DRAM (engines can't start at
                # unaligned partitions; DMA can).
                pv_stage = smpool.tile([H, HKV * D], f32, tag="pvstage")
                nc.scalar.copy(pv_stage[:, :512], pv_ps[:, :512])
                nc.vector.tensor_copy(pv_stage[:, 512:], pv_ps[:, 512:])
                for h in range(HKV):
                    nc.sync.dma_start(
                        out_d[s, G * h:G * (h + 1), :],
                        pv_stage[G * h:G * (h + 1), h * D:(h + 1) * D])

    nc.compile()
    return nc


def _get_nc():
    global _cached_nc
    if _cached_nc is None:
        _cached_nc = _build_nc()
    return _cached_nc


def _prep_shards(q, k, v, k_cache, v_cache, block_tables, context_lens,
                 slot_mapping):
    q = np.ascontiguousarray(np.asarray(q, np.float32))
    k = np.ascontiguousarray(np.asarray(k, np.float32))
    v = np.ascontiguousarray(np.asarray(v, np.float32))
    kc = np.asarray(k_cache, np.float32)
    vc = np.asarray(v_cache, np.float32)
    bt = np.asarray(block_tables)
    cl = np.asarray(context_lens, np.int32)

    expect = np.arange(S * MAX_BLOCKS, dtype=np.int64).reshape(S, MAX_BLOCKS)
    if not np.array_equal(np.asarray(bt, np.int64), expect):
        # General fallback (never hit for the spec's arange tables): gather
        # each sequence's blocks into contiguous order on the host.
        kc = kc[np.asarray(bt, np.int64)].reshape(S, T, HKV, D).reshape(
            S, T, HKV * D)
        vc = vc[np.asarray(bt, np.int64)].reshape(S, T, HKV, D).reshape(
            S, T, HKV * D)
    else:
        kc = kc.reshape(S, T, HKV * D)
        vc = vc.reshape(S, T, HKV * D)

    in_maps = []
    for c in range(NCORES):
        sl = slice(c * S_LOC, (c + 1) * S_LOC)
        in_maps.append({
            "q": q[sl],
            "k": k[sl],
            "v": v[sl],
            "kc": np.ascontiguousarray(kc[sl]),
            "vc": np.ascontiguousarray(vc[sl]),
            "cl": np.ascontiguousarray(cl[sl]).reshape(1, S_LOC),
        })
    return in_maps


def kernel(q, k, v, k_cache, v_cache, block_tables, context_lens,
           slot_mapping) -> np.ndarray:
    from concourse.bass_utils import run_bass_kernel_spmd

    nc = _get_nc()
    in_maps = _prep_shards(q, k, v, k_cache, v_cache, block_tables,
                           context_lens, slot_mapping)
    res = run_bass_kernel_spmd(nc, in_maps, core_ids=list(range(NCORES)),
                               trace=False)
    out = np.concatenate([res.results[c]["out"] for c in range(NCORES)],
                         axis=0)
    return np.ascontiguousarray(out.astype(np.float32))


# revision 32
# speedup vs baseline: 2.7182x; 2.7182x over previous
"""Paged-attention decode kernel for 8 TRN2 NeuronCores (SPMD, data-parallel over sequences).

Problem: nn_Attention_15659450761267 (sparse_attention).
  S=64 seqs, H=32 query heads, HKV=8 kv heads (GQA g=4), D=128, BS=16,
  MAX_BLOCKS=128, T=2048, f32 caches [8192,16,8,128].

Sharding: core c owns sequences [8c, 8c+8). block_tables is arange
(spec fill), so sequence s's cache lives in blocks [128s, 128(s+1)) ->
its K/V cache is a contiguous [2048, 1024] f32 slab. Each core reads
only its own 8 slabs (134 MB) -> memory-roofline ~375us/core.

The reference scatters the new-token k/v into the cache at slot cl-1,
then attends over positions < cl. Equivalently (softmax is permutation
invariant): attend over cached positions t < cl-1 (masking out the
stale slot cl-1) plus the new (k, v) appended as an extra column.
No device-side scatter needed.

Pipeline (per sequence, per 128-position chunk):
  K chunk --PE transpose--> KT --DVE copy--> SBUF (f32r)
  ST[t, (h,g)] = KT_h.T @ qt_h          (8 small fp32r matmuls -> one PSUM tile)
  p~ = exp(ST + mask_col)               (ONE ACT op, PSUM->SBUF, f32r out;
                                         mask col = -1e30 where pos >= cl-1)
  PV  += p~.T @ V_chunk                 (fp32r matmuls, N=512)
  sums += p~.T @ ones                   (softmax denominators via ones-column)
Then the new token is appended as a K=1 matmul, and the epilogue does
out = PV * (1/sums) in one fused DVE pass before band-DMAs to DRAM.
No max-subtraction is needed: scores are O(+-8) after SCALE.
"""

import numpy as np

S = 64
H = 32
HKV = 8
G = H // HKV  # 4
D = 128
BS = 16
MAX_BLOCKS = 128
T = MAX_BLOCKS * BS  # 2048
SCALE = 0.08838834764831845
NCORES = 8
S_LOC = S // NCORES  # 8
NEG = -1.0e30
CHUNK = 128          # positions per chunk (one transpose / ST tile)
NCHUNK = T // CHUNK  # 16
BLK = 512            # positions per K-load block
NBLK = T // BLK      # 4
CPB = BLK // CHUNK   # 4

_cached_nc = None


def _build_nc(reps=1):
    import concourse.mybir as mybir
    import concourse.tile as tile
    from concourse import bacc
    from concourse.masks import make_identity

    f32 = mybir.dt.float32
    f32r = mybir.dt.float32r
    i32 = mybir.dt.int32
    Alu = mybir.AluOpType
    Act = mybir.ActivationFunctionType

    nc = bacc.Bacc("TRN2", target_bir_lowering=False, debug=False,
                   num_devices=NCORES)
    q_d = nc.dram_tensor("q", [S_LOC, H, D], f32, kind="ExternalInput")
    k_d = nc.dram_tensor("k", [S_LOC, HKV, D], f32, kind="ExternalInput")
    v_d = nc.dram_tensor("v", [S_LOC, HKV, D], f32r, kind="ExternalInput")
    kc_d = nc.dram_tensor("kc", [S_LOC, T, HKV * D], f32r, kind="ExternalInput")
    vc_d = nc.dram_tensor("vc", [S_LOC, T, HKV * D], f32r, kind="ExternalInput")
    cl_d = nc.dram_tensor("cl", [1, S_LOC], i32, kind="ExternalInput")
    out_d = nc.dram_tensor("out", [S_LOC, H, D], f32, kind="ExternalOutput")

    with tile.TileContext(nc) as tc:
        with (
            tc.tile_pool(name="const", bufs=1) as constp,
            tc.tile_pool(name="kchunk", bufs=2) as kpool,
            tc.tile_pool(name="vchunk", bufs=2) as vpool,
            tc.tile_pool(name="kt", bufs=2) as ktpool,
            tc.tile_pool(name="stexp", bufs=4) as stpool,
            tc.tile_pool(name="small", bufs=2) as smpool,
            tc.tile_pool(name="ps_ktp", bufs=2, space="PSUM") as ps_ktp,
            tc.tile_pool(name="ps_st", bufs=2, space="PSUM") as ps_st,
            tc.tile_pool(name="ps_pv", bufs=1, space="PSUM") as ps_pv,
            tc.tile_pool(name="ps_sums", bufs=1, space="PSUM") as ps_sums,
            tc.tile_pool(name="ps_small", bufs=1, space="PSUM") as ps_small,
        ):
            ident = constp.tile([128, 128], f32)
            make_identity(nc, ident[:])
            identr = constp.tile([128, 128], f32r)
            nc.vector.tensor_copy(identr[:], ident[:])
            onesf = constp.tile([128, G], f32)
            nc.vector.memset(onesf[:], 1.0)
            ones_r = constp.tile([128, G], f32r)
            nc.vector.tensor_copy(ones_r[:], onesf[:])

            # posCols[p, j] = j*128 + p  (position of partition p in chunk j)
            posc_i = constp.tile([CHUNK, NCHUNK], i32)
            nc.gpsimd.iota(posc_i[:], pattern=[[CHUNK, NCHUNK]], base=0,
                           channel_multiplier=1)
            posc = constp.tile([CHUNK, NCHUNK], f32)
            nc.vector.tensor_copy(posc[:], posc_i[:])

            # context_lens -> f32 (cl - 1), broadcast over 128 partitions
            cli = constp.tile([1, S_LOC], i32)
            nc.sync.dma_start(cli[:], cl_d[:])
            clf = constp.tile([1, S_LOC], f32)
            nc.vector.tensor_copy(clf[:], cli[:])
            nc.vector.tensor_scalar_add(clf[:], clf[:], -1.0)
            clb = constp.tile([CHUNK, S_LOC], f32)
            nc.gpsimd.partition_broadcast(clb[:], clf[:])

            for s in [ss for _ in range(reps) for ss in range(S_LOC)]:
                # ---- q / new-token k,v ----
                q_sb = smpool.tile([H, D], f32, tag="q")
                nc.sync.dma_start(q_sb[:], q_d[s])
                kn_sb = smpool.tile([HKV, D], f32, tag="kn")
                nc.sync.dma_start(kn_sb[:], k_d[s])
                vn_sb = smpool.tile([1, HKV * D], f32r, tag="vn")
                nc.sync.dma_start(
                    vn_sb[:], v_d.rearrange("s h d -> s (h d)")[s][None, :])

                # QT = q^T * SCALE  [D, H] (f32r)
                qt_ps = ps_small.tile([D, H], f32, tag="misc")
                nc.tensor.transpose(qt_ps[:], q_sb[:], ident[:H, :H])
                qt_sb = smpool.tile([D, H], f32r, tag="qt")
                nc.scalar.mul(qt_sb[:], qt_ps[:], SCALE)

                kc_v = kc_d[s].rearrange("(c p) d -> p c d", p=CHUNK)
                vc_v = vc_d[s].rearrange("(c p) d -> p c d", p=CHUNK)

                pv_ps = ps_pv.tile([H, HKV * D], f32, tag="pv")
                sums_ps = ps_sums.tile([H, G], f32, tag="sums")
                for b in range(NBLK):
                    k_sb = kpool.tile([CHUNK, CPB, HKV * D], f32r,
                                      tag="kchunk")
                    nc.sync.dma_start(
                        k_sb[:], kc_v[:, b * CPB:(b + 1) * CPB, :])
                    v_sb = vpool.tile([CHUNK, CPB, HKV * D], f32r,
                                      tag="vchunk")
                    nc.sync.dma_start(
                        v_sb[:], vc_v[:, b * CPB:(b + 1) * CPB, :])

                    # K^T for this block: per head, 4 PE transposes into one
                    # PSUM bank, one wide DVE copy to SBUF (f32r).
                    kt = ktpool.tile([D, HKV, BLK], f32r, tag="kt")
                    for h in range(HKV):
                        ktp = ps_ktp.tile([D, BLK], f32r, tag="ktp")
                        for c2 in range(CPB):
                            nc.tensor.transpose(
                                ktp[:, c2 * CHUNK:(c2 + 1) * CHUNK],
                                k_sb[:, c2, h * D:(h + 1) * D], identr[:])
                        nc.vector.tensor_copy(kt[:, h], ktp[:])

                    for c2 in range(CPB):
                        c = b * CPB + c2
                        # ST[t, (h,g)] = k_t . q_(h,g) * SCALE (transposed!)
                        st_ps = ps_st.tile([CHUNK, H], f32, tag="st")
                        for h in range(HKV):
                            nc.tensor.matmul(
                                st_ps[:, G * h:G * (h + 1)],
                                kt[:, h, c2 * CHUNK:(c2 + 1) * CHUNK],
                                qt_sb[:, G * h:G * (h + 1)],
                                start=True, stop=True)
                        # mask column: -1e30 where position >= cl-1
                        mc = smpool.tile([CHUNK, 1], f32, tag="mc")
                        nc.vector.tensor_scalar(
                            mc[:], posc[:, c:c + 1], clb[:, s:s + 1], NEG,
                            op0=Alu.is_ge, op1=Alu.mult)
                        # p~ = exp(ST + mask): one ACT op, PSUM -> SBUF f32r
                        st_exp = stpool.tile([CHUNK, H], f32r, tag="stexp")
                        nc.scalar.activation(st_exp[:], st_ps[:], Act.Exp,
                                             bias=mc[:, 0:1])
                        # PV and denominator accumulation
                        first = (c == 0)
                        nc.tensor.matmul(pv_ps[:, :512], st_exp[:],
                                         v_sb[:, c2, :512],
                                         start=first, stop=False)
                        nc.tensor.matmul(pv_ps[:, 512:], st_exp[:],
                                         v_sb[:, c2, 512:],
                                         start=first, stop=False)
                        nc.tensor.matmul(sums_ps[:], st_exp[:], ones_r[:],
                                         start=first, stop=False)

                # ---- new token: p~_new row, appended as K=1 matmuls ----
                ktn_ps = ps_small.tile([D, HKV], f32, tag="misc")
                nc.tensor.transpose(ktn_ps[:], kn_sb[:], ident[:HKV, :HKV])
                ktn_sb = smpool.tile([D, HKV], f32r, tag="ktn_sb")
                nc.vector.tensor_copy(ktn_sb[:], ktn_ps[:])
                scn_st = smpool.tile([G, HKV], f32, tag="scnstage")
                for h in range(HKV):
                    scn_ps = ps_small.tile([G, HKV], f32, tag="misc")
                    nc.tensor.matmul(scn_ps[:], qt_sb[:, G * h:G * (h + 1)],
                                     ktn_sb[:], start=True, stop=True)
                    nc.vector.tensor_copy(scn_st[:, h:h + 1],
                                          scn_ps[:, h:h + 1])
                scn_exp = smpool.tile([G, HKV], f32, tag="scnexp")
                nc.scalar.activation(scn_exp[:], scn_st[:], Act.Exp)
                stn = smpool.tile([H, 1], f32, tag="stn")
                for h in range(HKV):
                    nc.scalar.dma_start(stn[G * h:G * (h + 1), :],
                                        scn_exp[:, h:h + 1])
                ptn_ps = ps_small.tile([1, H], f32, tag="misc")
                nc.tensor.transpose(ptn_ps[:], stn[:], ident[:H, :H])
                ptn_sb = smpool.tile([1, H], f32r, tag="ptn_sb")
                nc.vector.tensor_copy(ptn_sb[:], ptn_ps[:])
                nc.tensor.matmul(pv_ps[:, :512], ptn_sb[:], vn_sb[:, :512],
                                 start=False, stop=True)
                nc.tensor.matmul(pv_ps[:, 512:], ptn_sb[:], vn_sb[:, 512:],
                                 start=False, stop=True)
                nc.tensor.matmul(sums_ps[:], ptn_sb[:], ones_r[:1, :],
                                 start=False, stop=True)

                # ---- epilogue: out = PV / sums, band-DMA to DRAM ----
                sums_sb = smpool.tile([H, 1], f32, tag="sums_sb")
                nc.vector.tensor_copy(sums_sb[:], sums_ps[:, 0:1])
                rcp = smpool.tile([H, 1], f32, tag="rcp")
                nc.vector.reciprocal(rcp[:], sums_sb[:])
                pv_stage = smpool.tile([H, HKV * D], f32, tag="pvstage")
                nc.vector.tensor_scalar(pv_stage[:], pv_ps[:], rcp[:, 0:1],
                                        None, op0=Alu.mult)
                for h in range(HKV):
                    nc.scalar.dma_start(
                        out_d[s, G * h:G * (h + 1), :],
                        pv_stage[G * h:G * (h + 1), h * D:(h + 1) * D])

    nc.compile()
    return nc


def _get_nc():
    global _cached_nc
    if _cached_nc is None:
        _cached_nc = _build_nc()
    return _cached_nc


def _prep_shards(q, k, v, k_cache, v_cache, block_tables, context_lens,
                 slot_mapping):
    q = np.ascontiguousarray(np.asarray(q, np.float32))
    k = np.ascontiguousarray(np.asarray(k, np.float32))
    v = np.ascontiguousarray(np.asarray(v, np.float32))
    kc = np.asarray(k_cache, np.float32)
    vc = np.asarray(v_cache, np.float32)
    bt = np.asarray(block_tables)
    cl = np.asarray(context_lens, np.int32)

    expect = np.arange(S * MAX_BLOCKS, dtype=np.int64).reshape(S, MAX_BLOCKS)
    if not np.array_equal(np.asarray(bt, np.int64), expect):
        # General fallback (never hit for the spec's arange tables): gather
        # each sequence's blocks into contiguous order on the host.
        kc = kc[np.asarray(bt, np.int64)].reshape(S, T, HKV * D)
        vc = vc[np.asarray(bt, np.int64)].reshape(S, T, HKV * D)
    else:
        kc = kc.reshape(S, T, HKV * D)
        vc = vc.reshape(S, T, HKV * D)

    in_maps = []
    for c in range(NCORES):
        sl = slice(c * S_LOC, (c + 1) * S_LOC)
        in_maps.append({
            "q": q[sl],
            "k": k[sl],
            "v": v[sl],
            "kc": np.ascontiguousarray(kc[sl]),
            "vc": np.ascontiguousarray(vc[sl]),
            "cl": np.ascontiguousarray(cl[sl]).reshape(1, S_LOC),
        })
    return in_maps


def kernel(q, k, v, k_cache, v_cache, block_tables, context_lens,
           slot_mapping) -> np.ndarray:
    from concourse.bass_utils import run_bass_kernel_spmd

    nc = _get_nc()
    in_maps = _prep_shards(q, k, v, k_cache, v_cache, block_tables,
                           context_lens, slot_mapping)
    res = run_bass_kernel_spmd(nc, in_maps, core_ids=list(range(NCORES)),
                               trace=False)
    out = np.concatenate([res.results[c]["out"] for c in range(NCORES)],
                         axis=0)
    return np.ascontiguousarray(out.astype(np.float32))


# revision 34
# speedup vs baseline: 4.8647x; 1.7897x over previous
"""Paged-attention decode kernel for 8 TRN2 NeuronCores (SPMD, data-parallel over sequences).

Problem: nn_Attention_15659450761267 (sparse_attention).
  S=64 seqs, H=32 query heads, HKV=8 kv heads (GQA g=4), D=128, BS=16,
  MAX_BLOCKS=128, T=2048, f32 caches [8192,16,8,128].

Sharding: core c owns sequences [8c, 8c+8). block_tables is arange
(spec fill), so sequence s's cache lives in blocks [128s, 128(s+1)) ->
its K/V cache is a contiguous [2048, 1024] f32 slab. Each core reads
only its own 8 slabs (134 MB) -> memory-roofline ~375us/core.

The reference scatters the new-token k/v into the cache at slot cl-1,
then attends over positions < cl. Equivalently (softmax is permutation
invariant): attend over cached positions t < cl-1 (masking out the
stale slot cl-1) plus the new (k, v) appended as an extra column.
No device-side scatter needed.

Pipeline (per sequence, per 128-position chunk):
  K chunk --PE transpose--> KT --DVE copy--> SBUF (f32r)
  ST[t, (h,g)] = KT_h.T @ qt_h          (8 small fp32r matmuls -> one PSUM tile)
  p~ = exp(ST + mask_col)               (ONE ACT op, PSUM->SBUF, f32r out;
                                         mask col = -1e30 where pos >= cl-1)
  PV  += p~.T @ V_chunk                 (fp32r matmuls, N=512)
  sums += p~.T @ ones                   (softmax denominators via ones-column)
Then the new token is appended as a K=1 matmul, and the epilogue does
out = PV * (1/sums) in one fused DVE pass before band-DMAs to DRAM.
No max-subtraction is needed: scores are O(+-8) after SCALE.
"""

import numpy as np

S = 64
H = 32
HKV = 8
G = H // HKV  # 4
D = 128
BS = 16
MAX_BLOCKS = 128
T = MAX_BLOCKS * BS  # 2048
SCALE = 0.08838834764831845
NCORES = 8
S_LOC = S // NCORES  # 8
NEG = -1.0e30
CHUNK = 128          # positions per chunk (one transpose / ST tile)
NCHUNK = T // CHUNK  # 16
BLK = 512            # positions per K-load block
NBLK = T // BLK      # 4
CPB = BLK // CHUNK   # 4

_nc_cache = {}


def _build_nc(chunk_counts=(NCHUNK,) * S_LOC, reps=1):
    import concourse.mybir as mybir
    import concourse.tile as tile
    from concourse import bacc
    from concourse.masks import make_identity

    f32 = mybir.dt.float32
    f32r = mybir.dt.float32r
    i32 = mybir.dt.int32
    Alu = mybir.AluOpType
    Act = mybir.ActivationFunctionType

    nc = bacc.Bacc("TRN2", target_bir_lowering=False, debug=False,
                   num_devices=NCORES)
    q_d = nc.dram_tensor("q", [S_LOC, H, D], f32, kind="ExternalInput")
    k_d = nc.dram_tensor("k", [S_LOC, HKV, D], f32, kind="ExternalInput")
    v_d = nc.dram_tensor("v", [S_LOC, HKV, D], f32r, kind="ExternalInput")
    kc_d = nc.dram_tensor("kc", [S_LOC, T, HKV * D], f32r, kind="ExternalInput")
    vc_d = nc.dram_tensor("vc", [S_LOC, T, HKV * D], f32r, kind="ExternalInput")
    cl_d = nc.dram_tensor("cl", [1, S_LOC], i32, kind="ExternalInput")
    out_d = nc.dram_tensor("out", [S_LOC, H, D], f32, kind="ExternalOutput")

    with tile.TileContext(nc) as tc:
        with (
            tc.tile_pool(name="const", bufs=1) as constp,
            tc.tile_pool(name="kchunk", bufs=2) as kpool,
            tc.tile_pool(name="vchunk", bufs=2) as vpool,
            tc.tile_pool(name="kt", bufs=2) as ktpool,
            tc.tile_pool(name="stexp", bufs=4) as stpool,
            tc.tile_pool(name="small", bufs=2) as smpool,
            tc.tile_pool(name="ps_ktp", bufs=2, space="PSUM") as ps_ktp,
            tc.tile_pool(name="ps_st", bufs=2, space="PSUM") as ps_st,
            tc.tile_pool(name="ps_pv", bufs=1, space="PSUM") as ps_pv,
            tc.tile_pool(name="ps_sums", bufs=1, space="PSUM") as ps_sums,
            tc.tile_pool(name="ps_small", bufs=1, space="PSUM") as ps_small,
        ):
            ident = constp.tile([128, 128], f32)
            make_identity(nc, ident[:])
            identr = constp.tile([128, 128], f32r)
            nc.vector.tensor_copy(identr[:], ident[:])
            onesf = constp.tile([128, G], f32)
            nc.vector.memset(onesf[:], 1.0)
            ones_r = constp.tile([128, G], f32r)
            nc.vector.tensor_copy(ones_r[:], onesf[:])

            # posCols[p, j] = j*128 + p  (position of partition p in chunk j)
            posc_i = constp.tile([CHUNK, NCHUNK], i32)
            nc.gpsimd.iota(posc_i[:], pattern=[[CHUNK, NCHUNK]], base=0,
                           channel_multiplier=1)
            posc = constp.tile([CHUNK, NCHUNK], f32)
            nc.vector.tensor_copy(posc[:], posc_i[:])

            # context_lens -> f32 (cl - 1), broadcast over 128 partitions
            cli = constp.tile([1, S_LOC], i32)
            nc.sync.dma_start(cli[:], cl_d[:])
            clf = constp.tile([1, S_LOC], f32)
            nc.vector.tensor_copy(clf[:], cli[:])
            nc.vector.tensor_scalar_add(clf[:], clf[:], -1.0)
            clb = constp.tile([CHUNK, S_LOC], f32)
            nc.gpsimd.partition_broadcast(clb[:], clf[:])

            for s in [ss for _ in range(reps) for ss in range(S_LOC)]:
                # ---- q / new-token k,v ----
                q_sb = smpool.tile([H, D], f32, tag="q")
                nc.sync.dma_start(q_sb[:], q_d[s])
                kn_sb = smpool.tile([HKV, D], f32, tag="kn")
                nc.sync.dma_start(kn_sb[:], k_d[s])
                vn_sb = smpool.tile([1, HKV * D], f32r, tag="vn")
                nc.sync.dma_start(
                    vn_sb[:], v_d.rearrange("s h d -> s (h d)")[s][None, :])

                # QT = q^T * SCALE  [D, H] (f32r)
                qt_ps = ps_small.tile([D, H], f32, tag="misc")
                nc.tensor.transpose(qt_ps[:], q_sb[:], ident[:H, :H])
                qt_sb = smpool.tile([D, H], f32r, tag="qt")
                nc.scalar.mul(qt_sb[:], qt_ps[:], SCALE)

                kc_v = kc_d[s].rearrange("(c p) d -> p c d", p=CHUNK)
                vc_v = vc_d[s].rearrange("(c p) d -> p c d", p=CHUNK)

                pv_ps = ps_pv.tile([H, HKV * D], f32, tag="pv")
                sums_ps = ps_sums.tile([H, G], f32, tag="sums")
                nch = chunk_counts[s]
                nblocks = (nch + CPB - 1) // CPB
                for b in range(nblocks):
                    cpb = min(CPB, nch - b * CPB)
                    k_sb = kpool.tile([CHUNK, CPB, HKV * D], f32r,
                                      tag="kchunk")
                    nc.sync.dma_start(
                        k_sb[:, :cpb], kc_v[:, b * CPB:b * CPB + cpb, :])
                    v_sb = vpool.tile([CHUNK, CPB, HKV * D], f32r,
                                      tag="vchunk")
                    nc.sync.dma_start(
                        v_sb[:, :cpb], vc_v[:, b * CPB:b * CPB + cpb, :])

                    # K^T for this block: per head, PE transposes into one
                    # PSUM bank, one wide DVE copy to SBUF (f32r).
                    kt = ktpool.tile([D, HKV, BLK], f32r, tag="kt")
                    for h in range(HKV):
                        ktp = ps_ktp.tile([D, BLK], f32r, tag="ktp")
                        for c2 in range(cpb):
                            nc.tensor.transpose(
                                ktp[:, c2 * CHUNK:(c2 + 1) * CHUNK],
                                k_sb[:, c2, h * D:(h + 1) * D], identr[:])
                        nc.vector.tensor_copy(kt[:, h, :cpb * CHUNK],
                                              ktp[:, :cpb * CHUNK])

                    for c2 in range(cpb):
                        c = b * CPB + c2
                        # ST[t, (h,g)] = k_t . q_(h,g) * SCALE (transposed!)
                        st_ps = ps_st.tile([CHUNK, H], f32, tag="st")
                        for h in range(HKV):
                            nc.tensor.matmul(
                                st_ps[:, G * h:G * (h + 1)],
                                kt[:, h, c2 * CHUNK:(c2 + 1) * CHUNK],
                                qt_sb[:, G * h:G * (h + 1)],
                                start=True, stop=True)
                        # mask column: -1e30 where position >= cl-1
                        mc = smpool.tile([CHUNK, 1], f32, tag="mc")
                        nc.vector.tensor_scalar(
                            mc[:], posc[:, c:c + 1], clb[:, s:s + 1], NEG,
                            op0=Alu.is_ge, op1=Alu.mult)
                        # p~ = exp(ST + mask): one ACT op, PSUM -> SBUF f32r
                        st_exp = stpool.tile([CHUNK, H], f32r, tag="stexp")
                        nc.scalar.activation(st_exp[:], st_ps[:], Act.Exp,
                                             bias=mc[:, 0:1])
                        # PV and denominator accumulation
                        first = (c == 0)
                        nc.tensor.matmul(pv_ps[:, :512], st_exp[:],
                                         v_sb[:, c2, :512],
                                         start=first, stop=False)
                        nc.tensor.matmul(pv_ps[:, 512:], st_exp[:],
                                         v_sb[:, c2, 512:],
                                         start=first, stop=False)
                        nc.tensor.matmul(sums_ps[:], st_exp[:], ones_r[:],
                                         start=first, stop=False)

                # ---- new token: p~_new row, appended as K=1 matmuls ----
                ktn_ps = ps_small.tile([D, HKV], f32, tag="misc")
                nc.tensor.transpose(ktn_ps[:], kn_sb[:], ident[:HKV, :HKV])
                ktn_sb = smpool.tile([D, HKV], f32r, tag="ktn_sb")
                nc.vector.tensor_copy(ktn_sb[:], ktn_ps[:])
                scn_st = smpool.tile([G, HKV], f32, tag="scnstage")
                for h in range(HKV):
                    scn_ps = ps_small.tile([G, HKV], f32, tag="misc")
                    nc.tensor.matmul(scn_ps[:], qt_sb[:, G * h:G * (h + 1)],
                                     ktn_sb[:], start=True, stop=True)
                    nc.vector.tensor_copy(scn_st[:, h:h + 1],
                                          scn_ps[:, h:h + 1])
                scn_exp = smpool.tile([G, HKV], f32, tag="scnexp")
                nc.scalar.activation(scn_exp[:], scn_st[:], Act.Exp)
                stn = smpool.tile([H, 1], f32, tag="stn")
                for h in range(HKV):
                    nc.scalar.dma_start(stn[G * h:G * (h + 1), :],
                                        scn_exp[:, h:h + 1])
                ptn_ps = ps_small.tile([1, H], f32, tag="misc")
                nc.tensor.transpose(ptn_ps[:], stn[:], ident[:H, :H])
                ptn_sb = smpool.tile([1, H], f32r, tag="ptn_sb")
                nc.vector.tensor_copy(ptn_sb[:], ptn_ps[:])
                nc.tensor.matmul(pv_ps[:, :512], ptn_sb[:], vn_sb[:, :512],
                                 start=(nch == 0), stop=True)
                nc.tensor.matmul(pv_ps[:, 512:], ptn_sb[:], vn_sb[:, 512:],
                                 start=(nch == 0), stop=True)
                nc.tensor.matmul(sums_ps[:], ptn_sb[:], ones_r[:1, :],
                                 start=(nch == 0), stop=True)

                # ---- epilogue: out = PV / sums, band-DMA to DRAM ----
                sums_sb = smpool.tile([H, 1], f32, tag="sums_sb")
                nc.vector.tensor_copy(sums_sb[:], sums_ps[:, 0:1])
                rcp = smpool.tile([H, 1], f32, tag="rcp")
                nc.vector.reciprocal(rcp[:], sums_sb[:])
                pv_stage = smpool.tile([H, HKV * D], f32, tag="pvstage")
                nc.vector.tensor_scalar(pv_stage[:], pv_ps[:], rcp[:, 0:1],
                                        None, op0=Alu.mult)
                for h in range(HKV):
                    nc.scalar.dma_start(
                        out_d[s, G * h:G * (h + 1), :],
                        pv_stage[G * h:G * (h + 1), h * D:(h + 1) * D])

    nc.compile()
    return nc


def _get_nc(chunk_counts):
    key = tuple(chunk_counts)
    if key not in _nc_cache:
        _nc_cache[key] = _build_nc(chunk_counts=key)
    return _nc_cache[key]


def _plan(q, k, v, k_cache, v_cache, block_tables, context_lens,
          slot_mapping):
    """Sort sequences by context length, snake-deal to (core, slot), and
    compute per-slot static chunk counts (max over cores in each slot)."""
    q = np.ascontiguousarray(np.asarray(q, np.float32))
    k = np.ascontiguousarray(np.asarray(k, np.float32))
    v = np.ascontiguousarray(np.asarray(v, np.float32))
    kc = np.asarray(k_cache, np.float32)
    vc = np.asarray(v_cache, np.float32)
    bt = np.asarray(block_tables)
    cl = np.asarray(context_lens, np.int32)

    expect = np.arange(S * MAX_BLOCKS, dtype=np.int64).reshape(S, MAX_BLOCKS)
    if not np.array_equal(np.asarray(bt, np.int64), expect):
        # General fallback (never hit for the spec's arange tables): gather
        # each sequence's blocks into contiguous order on the host.
        kc = kc[np.asarray(bt, np.int64)].reshape(S, T, HKV * D)
        vc = vc[np.asarray(bt, np.int64)].reshape(S, T, HKV * D)
    else:
        kc = kc.reshape(S, T, HKV * D)
        vc = vc.reshape(S, T, HKV * D)

    # cached chunks needed for positions 0 .. cl-2
    need = np.ceil(np.maximum(cl - 1, 0) / CHUNK).astype(np.int64)
    order = np.argsort(-need, kind="stable")  # desc by need
    # snake deal: rank group j -> slot j; within group alternate direction
    assign = np.empty((NCORES, S_LOC), np.int64)
    for j in range(S_LOC):
        grp = order[j * NCORES:(j + 1) * NCORES]
        if j % 2 == 1:
            grp = grp[::-1]
        assign[:, j] = grp
    chunk_counts = tuple(int(need[assign[:, j]].max()) for j in range(S_LOC))

    in_maps = []
    for c in range(NCORES):
        idx = assign[c]
        in_maps.append({
            "q": np.ascontiguousarray(q[idx]),
            "k": np.ascontiguousarray(k[idx]),
            "v": np.ascontiguousarray(v[idx]),
            "kc": np.ascontiguousarray(kc[idx]),
            "vc": np.ascontiguousarray(vc[idx]),
            "cl": np.ascontiguousarray(cl[idx]).reshape(1, S_LOC),
        })
    return in_maps, assign, chunk_counts


def _prep_shards(q, k, v, k_cache, v_cache, block_tables, context_lens,
                 slot_mapping):
    in_maps, _, _ = _plan(q, k, v, k_cache, v_cache, block_tables,
                          context_lens, slot_mapping)
    return in_maps


def kernel(q, k, v, k_cache, v_cache, block_tables, context_lens,
           slot_mapping) -> np.ndarray:
    from concourse.bass_utils import run_bass_kernel_spmd

    in_maps, assign, chunk_counts = _plan(
        q, k, v, k_cache, v_cache, block_tables, context_lens, slot_mapping)
    nc = _get_nc(chunk_counts)
    res = run_bass_kernel_spmd(nc, in_maps, core_ids=list(range(NCORES)),
                               trace=False)
    out = np.empty((S, H, D), np.float32)
    for c in range(NCORES):
        out[assign[c]] = res.results[c]["out"]
    return np.ascontiguousarray(out)
